# revision 17
# baseline (speedup 1.0000x reference)
"""Trainium2 Bass kernel for nn_DecoderForGeoLossLess (3-level sparse
transposed-conv LSTM decoder, 5000 -> 20000 -> 80000 -> 320000 voxels).

v2 strategy (vs v1): eliminate the SWDGE dma_gather (measured 4.8us of Q7
descriptor-gen per 512 rows, 382us serialized) and deduplicate work.

- Dedup: rows with identical (parent, kofs) pairs are identical.  t1 level:
  80000 s-rows -> ~62k unique (r,k1) cells; t2 level: 320000 j-rows -> ~252k
  unique (s,k2) cells.  Host maps j -> cell and expands at the end.
- Phase A: per t1-cell, compute hx1/cx1 (t0 gates + t1 LSTM cell).  PE
  transposes write a slot-major parent table T in SBUF:
  T[slot%128, (slot//128)*128 + feat], feats = [hx1(64) | cx1(64)].
- Phase E: "gather" t2-cell inputs by PE one-hot expansion: for each window
  of 128 slots, matmul(stationary = T window [slots x feats], moving =
  host-baked one-hot [slots x cells]) -> PSUM [feats x cells], flushed to an
  SBUF subtable S in k2-bucket-major order.  Column ranges use a static
  per-(window,bucket) capacity grid (max across cores) so the single SPMD
  program works for all 8 cores; one-hot zero-columns yield unused pad cells.
- Phase B: per 1024-cell chunk: W_c matmul on S cx-rows, gates matmul on
  [hx1 ; x2] (x2 DMA'd into a fresh rhs tile), LSTM elementwise, bf16 out.
All matmuls bf16 operands with f32 PSUM accumulation.  No collectives.
"""

import os
import numpy as np
import ml_dtypes

import concourse.bass as bass
import concourse.mybir as mybir
import concourse.tile as tile
from concourse import bacc
from concourse.masks import make_identity
from concourse.bass_utils import run_bass_kernel_spmd

F32 = mybir.dt.float32
BF16 = mybir.dt.bfloat16
BF = ml_dtypes.bfloat16
SIG = mybir.ActivationFunctionType.Sigmoid
TANH = mybir.ActivationFunctionType.Tanh

N0, N1, N2, N3 = 5000, 20000, 80000, 320000
HID = 64
K = 8
NCORES = 8
CH = 512
PAIR = 2 * CH


def _round_up(x, m):
    return (x + m - 1) // m * m


def _assign_balanced(cellk, childk2, n):
    """Assign each t1-cell to a core, keeping per-(k1,k0)-group counts equal
    across cores, balancing per-(group, k2) child counts (these set the
    shared expansion-grid capacities) and global child totals.  Group slot
    capacities are window-aligned (multiples of 128)."""
    core_of = np.empty(n, np.int8)
    j_load = np.zeros(NCORES, np.float64)
    grp_cap = np.zeros(64, np.int64)
    tot = childk2.sum(axis=1)
    for cell in range(64):
        ss = np.nonzero(cellk == cell)[0]
        order = ss[np.argsort(-tot[ss], kind="stable")]
        percore = (len(ss) + NCORES - 1) // NCORES
        grp_cap[cell] = max(percore, 1)
        load = np.zeros((NCORES, K), np.float64)
        cnt = np.zeros(NCORES, np.int64)
        for s in order:
            v = childk2[s]
            sc = ((load + v) ** 2).sum(axis=1) + 1e-6 * j_load
            sc[cnt >= percore] = np.inf
            c = int(np.argmin(sc))
            core_of[s] = c
            load[c] += v
            cnt[c] += 1
            j_load[c] += tot[s]
    return core_of, grp_cap


def _dedupe_ldweights(nc):
    """Remove redundant consecutive InstLdweights (identical stationary
    AP/tile placement, no waits, no sem updates) from the finalized program.
    The PE keeps its loaded weights across matmuls, so a reload of the same
    weights is pure overhead (~125ns each); tile_legalize emits one per
    matmul unconditionally."""
    import concourse.mybir as mb

    def sig(i):
        w = i.ins[0]
        return (getattr(w, "memref", None), getattr(w, "offset", None),
                str(getattr(w, "ap", None)), str(getattr(w, "dtype", None)),
                i.tile_position, i.tile_size, i.perf_mode, i.is_transpose)

    removed = 0
    for f in nc.m.functions:
        for blk in f.blocks:
            insts = blk.instructions
            keep = []
            last = None
            for i in insts:
                if isinstance(i, mb.InstLdweights):
                    si = i.sync_info
                    clean = si is None or (not si.on_wait and not si.on_update)
                    s = sig(i)
                    if clean and last is not None and s == last:
                        removed += 1
                        continue
                    last = s
                keep.append(i)
            if len(keep) != len(insts):
                blk.instructions = keep
    return removed


def _segments(bounds_k, lo, hi):
    """Static segment list [(st, en, k)] intersecting [lo, hi) with the
    monotone boundary table bounds_k = list of (end_pos, k, start_pos)."""
    segs = []
    for end_pos, kk, start_pos in bounds_k:
        st = max(lo, start_pos)
        en = min(hi, end_pos)
        if st < en:
            segs.append((st - lo, en - lo, kk))
    return segs


def _prepare(x0, x1, x2, W_i, W_h, W_c,
             parent0, kofs0, parent1, kofs1, parent2, kofs2):
    s_par = parent2.astype(np.int64)
    k2j = kofs2.astype(np.int64)
    r_of_s = parent1.astype(np.int64)
    k1s = kofs1.astype(np.int64)
    q_of_r = parent0.astype(np.int64)
    k0r = kofs0.astype(np.int64)

    # ---- dedup t2: unique (s, k2) cells; j -> cell
    cell2 = s_par * K + k2j
    uc2, inv_j = np.unique(cell2, return_inverse=True)
    s2 = uc2 // K
    k2c = (uc2 % K).astype(np.int64)
    n2c = len(uc2)

    # ---- dedup t1: unique (r, k1) cells among referenced s
    cell1_of_s = r_of_s * K + k1s
    uc1 = np.unique(cell1_of_s[np.unique(s2)])
    r1 = uc1 // K
    k11 = uc1 % K
    n1c = len(uc1)
    pcell = np.searchsorted(uc1, cell1_of_s[s2])      # t2cell -> t1cell
    childk2 = np.zeros((n1c, K), np.int64)
    np.add.at(childk2, (pcell, k2c), 1)

    # ---- core assignment of t1-cells, balanced per (k1,k0)-group
    cellk = k11 * K + k0r[r1]
    core_of_t1, cell_pad = _assign_balanced(cellk, childk2, n1c)
    NS_real = int(cell_pad.sum())
    NS_pad = _round_up(NS_real, PAIR)
    cell_pad[63] += NS_pad - NS_real
    cell_off = np.concatenate([[0], np.cumsum(cell_pad)])

    spos = np.zeros(n1c, np.int64)
    t1_slot_lists = []
    for c in range(NCORES):
        slots_all, id_all = [], []
        for cell in range(64):
            ss = np.nonzero((cellk == cell) & (core_of_t1 == c))[0]
            slots = np.arange(cell_off[cell], cell_off[cell] + len(ss))
            spos[ss] = slots
            slots_all.append(slots)
            id_all.append(ss)
        t1_slot_lists.append((np.concatenate(slots_all), np.concatenate(id_all)))

    # ---- static (window, bucket) capacity grid, shared across cores
    W = NS_pad // 128
    core2 = core_of_t1[pcell]
    wof = spos[pcell] // 128                          # window of each t2cell
    cnt_wb = np.zeros((NCORES, W, K), np.int64)
    for c in range(NCORES):
        m = core2 == c
        np.add.at(cnt_wb[c], (wof[m], k2c[m]), 1)
    cap_wb = cnt_wb.max(axis=0)                       # [W, K]
    b2 = cap_wb.sum(axis=0)
    NJ_real = int(b2.sum())
    NJ_pad = _round_up(NJ_real, PAIR)
    cap_wb[W - 1, K - 1] += NJ_pad - NJ_real
    b2[K - 1] += NJ_pad - NJ_real
    boff = np.concatenate([[0], np.cumsum(b2)])
    # grid col of (w, b) run start
    goff = boff[:K][None, :] + (np.cumsum(cap_wb, axis=0) - cap_wb)

    # ---- per-core placement: subtable cols, one-hot, X2T
    sub_of_cell = np.zeros(n2c, np.int64)
    in_maps = []
    for c in range(NCORES):
        OH = np.zeros((128, NJ_pad), BF)
        X2T = np.zeros((64, NJ_pad), BF)
        for b in range(K):
            ids = np.nonzero((core2 == c) & (k2c == b))[0]
            ps = spos[pcell[ids]]
            order = np.argsort(ps, kind="stable")     # (window, slot) sorted
            ids = ids[order]
            ps = ps[order]
            ws = ps // 128
            # rank within each (w, b) group -> static grid position
            # groups are contiguous runs in ws (sorted)
            uw, st_idx = np.unique(ws, return_index=True)
            ends = np.append(st_idx[1:], len(ws))
            for wv, si, ei in zip(uw, st_idx, ends):
                sc = goff[wv, b] + np.arange(ei - si)
                sub_of_cell[ids[si:ei]] = sc
                OH[ps[si:ei] % 128, sc] = np.float32(1.0)
                X2T[:, sc] = x2[s2[ids[si:ei]]].astype(BF).T

        slots, tids = t1_slot_lists[c]
        X0T = np.zeros((64, NS_pad), BF)
        X0T[:, slots] = x0[q_of_r[r1[tids]]].astype(BF).T
        X1T = np.zeros((64, NS_pad), BF)
        X1T[:, slots] = x1[r1[tids]].astype(BF).T
        in_maps.append({"X0T": X0T, "X1T": X1T, "X2T": X2T,
                        "OH": np.ascontiguousarray(OH)})

    # ---- packed weights (shared); per-k gate order [i, c, o] / [i, c, f, o]
    # original split order: in 0:64, f 64:128, c 128:192, o 192:256
    perm = np.concatenate([np.arange(0, 64), np.arange(128, 192),
                           np.arange(64, 128), np.arange(192, 256)])
    W0 = np.zeros((64, K * 192), BF)
    W1 = np.zeros((128, K * 256), BF)
    W1s = np.zeros((128, K * 256), BF)
    WC = np.zeros((128, K * 64), BF)
    for k in range(K):
        wi = W_i[k].astype(BF)
        wh = W_h[k].astype(BF)
        W0[:, 192 * k:192 * k + 64] = wi[:, 0:64]
        W0[:, 192 * k + 64:192 * k + 128] = wi[:, 128:192]
        W0[:, 192 * k + 128:192 * k + 192] = wi[:, 192:256]
        W1[0:64, 256 * k:256 * (k + 1)] = wh[:, perm]
        W1[64:128, 256 * k:256 * (k + 1)] = wi[:, perm]
        W1s[0:64, 256 * k:256 * (k + 1)] = wi[:, perm]
        W1s[64:128, 256 * k:256 * (k + 1)] = wh[:, perm]
        WC[0:64, 64 * k:64 * (k + 1)] = W_c[k].astype(BF)
        WC[64:128, 64 * k:64 * (k + 1)] = W_c[k].astype(BF)
    for m in in_maps:
        m.update({"W0": W0, "W1": W1, "W1s": W1s, "WC": WC})

    # ---- baked segment tables
    cellb = [(int(cell_off[cg + 1]), (cg % K, cg // K), int(cell_off[cg]))
             for cg in range(64)]                  # (end, (k0, k1), start)
    t0b = [(e, kk[0], s) for (e, kk, s) in cellb]
    t1b = []
    for k1 in range(K):
        t1b.append((int(cell_off[(k1 + 1) * K]), k1, int(cell_off[k1 * K])))
    t2b = [(int(boff[b + 1]), b, int(boff[b])) for b in range(K)]

    meta = dict(NS_pad=NS_pad, NJ_pad=NJ_pad, t0b=t0b, t1b=t1b, t2b=t2b,
                cap_wb=cap_wb, goff=goff, boff=boff,
                sub_of_cell=sub_of_cell, core2=core2, inv_j=inv_j)
    return in_maps, meta


def _build(meta):
    NS_pad, NJ_pad = meta["NS_pad"], meta["NJ_pad"]
    t0b, t1b, t2b = meta["t0b"], meta["t1b"], meta["t2b"]
    cap_wb, goff, boff = meta["cap_wb"], meta["goff"], meta["boff"]
    NW = NS_pad // 128

    nc = bacc.Bacc("TRN2", target_bir_lowering=False, debug=False,
                   num_devices=NCORES)
    X0T = nc.dram_tensor("X0T", [64, NS_pad], BF16, kind="ExternalInput")
    X1T = nc.dram_tensor("X1T", [64, NS_pad], BF16, kind="ExternalInput")
    X2T = nc.dram_tensor("X2T", [64, NJ_pad], BF16, kind="ExternalInput")
    OH = nc.dram_tensor("OH", [128, NJ_pad], BF16, kind="ExternalInput")
    W0 = nc.dram_tensor("W0", [64, K * 192], BF16, kind="ExternalInput")
    W1 = nc.dram_tensor("W1", [128, K * 256], BF16, kind="ExternalInput")
    W1s = nc.dram_tensor("W1s", [128, K * 256], BF16, kind="ExternalInput")
    WC = nc.dram_tensor("WC", [128, K * 64], BF16, kind="ExternalInput")
    OUT = nc.dram_tensor("OUT", [128, NJ_pad // 2], BF16, kind="ExternalOutput")

    _ph = os.environ.get("KPHASE", "AEB")
    nA = NS_pad // PAIR if "A" in _ph else 0
    doE = "E" in _ph
    nB = NJ_pad // PAIR if "B" in _ph else 0

    with tile.TileContext(nc) as tc:
        with (
            tc.tile_pool(name="const", bufs=1) as cst,
            tc.tile_pool(name="sa", bufs=2) as sa,
            tc.tile_pool(name="sbp", bufs=2) as sbp,
        ):
            w0 = cst.tile([64, K * 192], BF16)
            nc.sync.dma_start(w0[:], W0[:])
            w1 = cst.tile([128, K * 256], BF16)
            nc.sync.dma_start(w1[:], W1[:])
            w1s = cst.tile([128, K * 256], BF16)
            nc.sync.dma_start(w1s[:], W1s[:])
            wc = cst.tile([128, K * 64], BF16)
            nc.sync.dma_start(wc[:], WC[:])
            idt = cst.tile([128, 64], BF16)
            make_identity(nc, idt[0:64, :])
            make_identity(nc, idt[64:128, :])

            tblT = cst.tile([128, NS_pad], BF16)     # parent table (slot-major)
            subS = cst.tile([128, NJ_pad], BF16)     # expanded cell table

            # ---------------- phase A1: t0 cells --------------------------
            cx0s, r1as, r1bs = [], [], []
            with (
                tc.tile_pool(name="pt0", bufs=2, space="PSUM") as pt0,
                tc.tile_pool(name="pt0o", bufs=2, space="PSUM") as pt0o,
            ):
                for p in range(nA):
                    w_lo = p * PAIR
                    x0p = sa.tile([64, PAIR], BF16, tag="x0p")
                    nc.sync.dma_start(x0p[:], X0T[:, w_lo:w_lo + PAIR])

                    t0ic = pt0.tile([128, PAIR], F32, tag="pt0")
                    t0o = pt0o.tile([128, CH], F32, tag="pt0o")
                    for side in range(2):
                        pb = 64 * side
                        segs = _segments(t0b, w_lo + side * CH,
                                         w_lo + (side + 1) * CH)
                        for st, en, k0 in segs:
                            rhs = x0p[:, side * CH + st:side * CH + en]
                            nc.tensor.matmul(
                                t0ic[pb:pb + 64, st:en],
                                w0[:, 192 * k0:192 * k0 + 64],
                                rhs, start=True, stop=True,
                                tile_position=(0, pb))
                            nc.tensor.matmul(
                                t0ic[pb:pb + 64, CH + st:CH + en],
                                w0[:, 192 * k0 + 64:192 * k0 + 128],
                                rhs, start=True, stop=True,
                                tile_position=(0, pb))
                            nc.tensor.matmul(
                                t0o[pb:pb + 64, st:en],
                                w0[:, 192 * k0 + 128:192 * k0 + 192],
                                rhs, start=True, stop=True,
                                tile_position=(0, pb))
                    s0 = sa.tile([128, PAIR], BF16, tag="s0")
                    nc.scalar.activation(s0[:], t0ic[:], SIG)
                    so0 = sa.tile([128, CH], BF16, tag="so0")
                    nc.scalar.activation(so0[:], t0o[:], SIG)
                    cx0 = sa.tile([128, CH], BF16, tag="cx0", bufs=nA,
                                  name=f"cx0_{p}")
                    nc.vector.tensor_mul(cx0[:], s0[:, 0:CH], s0[:, CH:PAIR])
                    t0t = sa.tile([128, CH], BF16, tag="t0t")
                    nc.scalar.activation(t0t[:], cx0[:], TANH)

                    r1a = sa.tile([128, CH], BF16, tag="r1a", bufs=nA,
                                  name=f"r1a_{p}")
                    nc.sync.dma_start(r1a[64:128, :], X1T[:, w_lo:w_lo + CH])
                    nc.vector.tensor_mul(r1a[0:64, :], so0[0:64, :],
                                         t0t[0:64, :])
                    r1b = sa.tile([128, CH], BF16, tag="r1b", bufs=nA,
                                  name=f"r1b_{p}")
                    nc.sync.dma_start(r1b[0:64, :],
                                      X1T[:, w_lo + CH:w_lo + PAIR])
                    nc.vector.tensor_mul(r1b[64:128, :], so0[64:128, :],
                                         t0t[64:128, :])
                    cx0s.append(cx0)
                    r1as.append(r1a)
                    r1bs.append(r1b)

            # ---------------- phase A2: t1 cells -> table -----------------
            with (
                tc.tile_pool(name="pt1a", bufs=2, space="PSUM") as pt1a,
                tc.tile_pool(name="pt1b", bufs=1, space="PSUM") as pt1b,
                tc.tile_pool(name="pshr", bufs=1, space="PSUM") as pshr,
            ):
                for p in range(nA):
                    w_lo = p * PAIR
                    cx0, r1a, r1b = cx0s[p], r1as[p], r1bs[p]
                    t1ic = pt1a.tile([128, PAIR], F32, tag="pt1a")
                    t1fo = pt1b.tile([128, PAIR], F32, tag="pt1b")
                    cxu = pshr.tile([128, CH], F32, tag="pshr")
                    for side in range(2):
                        pb = 64 * side
                        r1 = r1a if side == 0 else r1b
                        wg = w1 if side == 0 else w1s
                        segs = _segments(t1b, w_lo + side * CH,
                                         w_lo + (side + 1) * CH)
                        for st, en, k1 in segs:
                            rhs = r1[:, st:en]
                            nc.tensor.matmul(
                                t1ic[pb:pb + 64, st:en],
                                wg[:, 256 * k1:256 * k1 + 64],
                                rhs, start=True, stop=True,
                                tile_position=(0, pb))
                            nc.tensor.matmul(
                                t1ic[pb:pb + 64, CH + st:CH + en],
                                wg[:, 256 * k1 + 64:256 * k1 + 128],
                                rhs, start=True, stop=True,
                                tile_position=(0, pb))
                            nc.tensor.matmul(
                                t1fo[pb:pb + 64, st:en],
                                wg[:, 256 * k1 + 128:256 * k1 + 192],
                                rhs, start=True, stop=True,
                                tile_position=(0, pb))
                            nc.tensor.matmul(
                                t1fo[pb:pb + 64, CH + st:CH + en],
                                wg[:, 256 * k1 + 192:256 * k1 + 256],
                                rhs, start=True, stop=True,
                                tile_position=(0, pb))
                            nc.tensor.matmul(
                                cxu[pb:pb + 64, st:en],
                                wc[pb:pb + 64, 64 * k1:64 * (k1 + 1)],
                                cx0[pb:pb + 64, st:en], start=True, stop=True,
                                tile_position=(pb, pb))
                    s1 = sa.tile([128, PAIR], BF16, tag="s1")
                    nc.scalar.activation(s1[:], t1ic[:], SIG)
                    s1b = sa.tile([128, PAIR], BF16, tag="s1b")
                    nc.scalar.activation(s1b[:], t1fo[:], SIG)
                    ppr = sa.tile([128, CH], BF16, tag="ppr")
                    nc.vector.tensor_mul(ppr[:], s1[:, 0:CH], s1[:, CH:PAIR])
                    qq = sa.tile([128, CH], BF16, tag="qq")
                    nc.vector.tensor_mul(qq[:], s1b[:, 0:CH], cxu[:])
                    cxt = sa.tile([128, CH], BF16, tag="cxt")
                    nc.vector.tensor_add(cxt[:], ppr[:], qq[:])
                    t1t = sa.tile([128, CH], BF16, tag="t1t")
                    nc.scalar.activation(t1t[:], cxt[:], TANH)
                    hx1 = sa.tile([128, CH], BF16, tag="hx1")
                    nc.vector.tensor_mul(hx1[:], s1b[:, CH:PAIR], t1t[:])

                    # transpose 8 slot-blocks into the table
                    for b8 in range(8):
                        side = b8 // 4
                        cb = b8 % 4
                        pb = 64 * side
                        pt = pshr.tile([128, 128], BF16, tag="pshr", name="pt")
                        tp = (pb, 0) if side else None
                        nc.tensor.transpose(
                            pt[:, 0:64],
                            hx1[pb:pb + 64, 128 * cb:128 * (cb + 1)],
                            idt[pb:pb + 64, :], tile_position=tp)
                        nc.tensor.transpose(
                            pt[:, 64:128],
                            cxt[pb:pb + 64, 128 * cb:128 * (cb + 1)],
                            idt[pb:pb + 64, :], tile_position=tp)
                        blk = (p * 8 + b8) * 128
                        nc.vector.tensor_copy(tblT[:, blk:blk + 128], pt[:])

            # ---------------- phase E: one-hot expansion ----------------
            if doE:
                with (
                    tc.tile_pool(name="pse", bufs=1, space="PSUM") as pse,
                    tc.tile_pool(name="poh", bufs=2) as poh,
                ):
                    bank_lo = [int(boff[b]) for b in range(K)]
                    bank_sz = [0] * K
                    fill = [0] * K
                    ps_t = [None] * K
                    oh_t = [None] * K
                    for w in range(NW):
                        win = tblT[:, 128 * w:128 * (w + 1)]
                        for b in range(K):
                            cap = int(cap_wb[w, b])
                            if cap == 0:
                                continue
                            cur = int(goff[w, b])
                            rem = cap
                            while rem:
                                if ps_t[b] is None:
                                    bank_sz[b] = min(CH, int(boff[b + 1]) - bank_lo[b])
                                    ps_t[b] = pse.tile([128, CH], F32,
                                                       tag=f"eb{b}",
                                                       name=f"eps{b}")
                                    oh_t[b] = poh.tile([128, CH], BF16,
                                                       tag=f"oh{b}",
                                                       name=f"oht{b}")
                                    nc.sync.dma_start(
                                        oh_t[b][:, 0:bank_sz[b]],
                                        OH[:, bank_lo[b]:bank_lo[b] + bank_sz[b]])
                                    fill[b] = 0
                                take = min(rem, bank_lo[b] + bank_sz[b] - cur)
                                rel = cur - bank_lo[b]
                                nc.tensor.matmul(
                                    ps_t[b][:, rel:rel + take],
                                    win, oh_t[b][:, rel:rel + take],
                                    start=True, stop=True)
                                cur += take
                                rem -= take
                                fill[b] += take
                                if fill[b] == bank_sz[b]:
                                    # flush: alternate DVE / ScalarE (both
                                    # idle during the PE-bound expansion)
                                    dst = subS[:, bank_lo[b]:bank_lo[b] + bank_sz[b]]
                                    src = ps_t[b][:, 0:bank_sz[b]]
                                    if (w + b) % 4 == 1 and os.environ.get("KFLUSH", "1") == "1":
                                        nc.scalar.copy(dst, src)
                                    else:
                                        nc.vector.tensor_copy(dst, src)
                                    bank_lo[b] += bank_sz[b]
                                    ps_t[b] = None

            # ---------------- phase B: t2 -------------------------------
            with (
                tc.tile_pool(name="pga", bufs=2, space="PSUM") as pga,
                tc.tile_pool(name="pgb", bufs=1, space="PSUM") as pgb,
                tc.tile_pool(name="pcb", bufs=2, space="PSUM") as pcb,
            ):
                for p in range(nB):
                    w_lo = p * PAIR
                    segs2 = [_segments(t2b, w_lo + s * CH, w_lo + (s + 1) * CH)
                             for s in range(2)]

                    cxu2p = pcb.tile([128, CH], F32, tag="pcb")
                    for side in range(2):
                        pb = 64 * side
                        for st, en, k2 in segs2[side]:
                            nc.tensor.matmul(
                                cxu2p[pb:pb + 64, st:en],
                                wc[64:128, 64 * k2:64 * (k2 + 1)],
                                subS[64:128,
                                     w_lo + side * CH + st:w_lo + side * CH + en],
                                start=True, stop=True, tile_position=(64, pb))
                    rhs2 = sbp.tile([128, PAIR], BF16, tag="rhs2")
                    nc.vector.tensor_copy(rhs2[0:64, :],
                                          subS[0:64, w_lo:w_lo + PAIR])
                    nc.sync.dma_start(rhs2[64:128, :], X2T[:, w_lo:w_lo + PAIR])

                    p2a = pga.tile([128, PAIR], F32, tag="pga")
                    p2b = pgb.tile([128, PAIR], F32, tag="pgb")
                    for side in range(2):
                        pb = 64 * side
                        for st, en, k2 in segs2[side]:
                            rhs = rhs2[:, side * CH + st:side * CH + en]
                            nc.tensor.matmul(
                                p2a[pb:pb + 64, st:en],
                                w1[:, 256 * k2:256 * k2 + 64],
                                rhs, start=True, stop=True,
                                tile_position=(0, pb))
                            nc.tensor.matmul(
                                p2a[pb:pb + 64, CH + st:CH + en],
                                w1[:, 256 * k2 + 64:256 * k2 + 128],
                                rhs, start=True, stop=True,
                                tile_position=(0, pb))
                            nc.tensor.matmul(
                                p2b[pb:pb + 64, st:en],
                                w1[:, 256 * k2 + 128:256 * k2 + 192],
                                rhs, start=True, stop=True,
                                tile_position=(0, pb))
                            nc.tensor.matmul(
                                p2b[pb:pb + 64, CH + st:CH + en],
                                w1[:, 256 * k2 + 192:256 * k2 + 256],
                                rhs, start=True, stop=True,
                                tile_position=(0, pb))
                    s2 = sbp.tile([128, PAIR], BF16, tag="s2")
                    nc.scalar.activation(s2[:], p2a[:], SIG)
                    s2b = sbp.tile([128, PAIR], BF16, tag="s2b")
                    nc.scalar.activation(s2b[:], p2b[:], SIG)
                    _gp = os.environ.get("KGP", "0") == "1"
                    meng = nc.gpsimd if _gp else nc.vector
                    ppr2 = sbp.tile([128, CH], BF16, tag="ppr2")
                    meng.tensor_mul(ppr2[:], s2[:, 0:CH], s2[:, CH:PAIR])
                    qq2 = sbp.tile([128, CH], BF16, tag="qq2")
                    nc.vector.tensor_mul(qq2[:], s2b[:, 0:CH], cxu2p[:])
                    cxf = sbp.tile([128, CH], BF16, tag="cxf")
                    nc.vector.tensor_add(cxf[:], ppr2[:], qq2[:])
                    t2t = sbp.tile([128, CH], BF16, tag="t2t")
                    nc.scalar.activation(t2t[:], cxf[:], TANH)
                    hxo = sbp.tile([128, CH], BF16, tag="hxo")
                    meng.tensor_mul(hxo[:], s2b[:, CH:PAIR], t2t[:])
                    nc.sync.dma_start(OUT[:, w_lo // 2:w_lo // 2 + CH], hxo[:])

    nc.finalize()
    if os.environ.get("KSURG", "1") == "1":
        _dedupe_ldweights(nc)
    return nc


def _run(inputs, trace=False):
    in_maps, meta = _prepare(**inputs)
    nc = _build(meta)
    res = run_bass_kernel_spmd(nc, in_maps, core_ids=list(range(NCORES)),
                               trace=trace)
    NJ_pad = meta["NJ_pad"]
    sub_of_cell = meta["sub_of_cell"]
    core2 = meta["core2"]
    inv_j = meta["inv_j"]
    n2c = len(sub_of_cell)
    cellvals = np.zeros((n2c, 64), np.float32)
    for c in range(NCORES):
        oc = np.asarray(res.results[c]["OUT"]).astype(np.float32)
        # col q*512+t at rows [h*64:(h+1)*64] holds subtable col q*1024+h*512+t
        flat = oc.reshape(2, 64, NJ_pad // PAIR, CH).transpose(1, 2, 0, 3)
        flat = flat.reshape(64, NJ_pad)
        m = core2 == c
        cellvals[m] = flat[:, sub_of_cell[m]].T
    out = cellvals[inv_j]
    return out, res


def kernel(**inputs):
    out, _ = _run(inputs, trace=False)
    return out
